# revision 7
# baseline (speedup 1.0000x reference)
"""Gaussian upsampling module on 8 Trainium2 cores.

Full-input contract: kernel(**inputs) takes the complete (B=32, L=512, D=512)
problem, shards batches 4-per-core across 8 NeuronCores, runs one Bass/Tile
SPMD kernel, and reassembles the full outputs (x_upsamp (B,T,D), weights
(B,L,T)).

Math (per batch b):
  xs   = x + conv1d(energies) + conv1d(pitch)                  (L, D)
  lin  = (xs + conv1d(durations_float)) @ w_lin.T + b_lin      (L,)
  r    = softplus(lin)  [invalid l -> effectively masked]
  m    = cumsum(d) - 0.5*d,  d = durations_int
  probs[l,t] = exp(-0.5*((t+0.5-m_l)/r_l)^2 - ln r_l - 0.5*ln(2pi)), 0 if invalid
  weights    = probs / (sum_l probs + 1e-20)
  x_upsamp   = einsum('ld,lt->td', xs, weights)

Device mapping:
  - conv1d collapses to a K=7 matmul (6 shifted signal rows + ones row).
  - the dur-conv contribution to lin collapses to 4 scalars (computed on
    device with tiny matmuls) applied to shifted duration rows.
  - cumsum over L via matmul with an upper-triangular constant.
  - scores: ACT Square(frames - m) then ACT Exp(sq*a + c) with per-partition
    scale/bias; masking folded into c (-1e30) and lin (+1e4).
  - denominator: ones-column matmul over the partition (L) axis.
  - einsum: f32r matmuls, weights tiles as lhsT, xs as rhs.

Column layouts use free index f = c*BPC + b (chunk-major) so per-chunk
slices are contiguous.
"""
import numpy as np
from contextlib import ExitStack

import concourse.bass as bass
import concourse.mybir as mybir
import concourse.tile as tile
from concourse import bacc
from concourse.bass_utils import run_bass_kernel_spmd

F32 = mybir.dt.float32
F32R = mybir.dt.float32r
I32 = mybir.dt.int32
AF = mybir.ActivationFunctionType
OP = mybir.AluOpType
AX = mybir.AxisListType

B, L, D = 32, 512, 512
NCORES = 8
BPC = B // NCORES          # batches per core
NCH = L // 128             # 128-partition L-chunks
LOG_2PI = float(np.log(2.0 * np.pi))

_NC_CACHE = {}


def _ts(i, n):
    return slice(i * n, (i + 1) * n)


def _build(T):
    """Build + compile the per-core kernel for nb_frames == T."""
    nc = bacc.Bacc("TRN2", target_bir_lowering=False, debug=False)

    # T-tiles of <=512 along the frame axis, and <=128 M-chunks for the einsum
    tts = [(s, min(512, T - s)) for s in range(0, T, 512)]
    mms = [(s, min(128, T - s)) for s in range(0, T, 128)]

    # ---- DRAM I/O -----------------------------------------------------------
    x_in = nc.dram_tensor("x_loc", [BPC, L, D], F32, kind="ExternalInput").ap()
    en_in = nc.dram_tensor("en_pad", [BPC, L + 2], F32, kind="ExternalInput").ap()
    pi_in = nc.dram_tensor("pi_pad", [BPC, L + 2], F32, kind="ExternalInput").ap()
    df_in = nc.dram_tensor("df_pad", [BPC, L + 2], F32, kind="ExternalInput").ap()
    di_in = nc.dram_tensor("di_loc", [BPC, L], I32, kind="ExternalInput").ap()
    lens_in = nc.dram_tensor("lens_loc", [1, BPC], I32, kind="ExternalInput").ap()
    w6_in = nc.dram_tensor("w6", [6, D], F32, kind="ExternalInput").ap()
    ben_in = nc.dram_tensor("ben", [1, D], F32, kind="ExternalInput").ap()
    bpi_in = nc.dram_tensor("bpi", [1, D], F32, kind="ExternalInput").ap()
    wdb_in = nc.dram_tensor("wdb_c", [128, NCH, 4], F32, kind="ExternalInput").ap()
    wlinc_in = nc.dram_tensor("wlin_c", [128, NCH], F32, kind="ExternalInput").ap()
    wlinr_in = nc.dram_tensor("wlin_r", [1, D], F32, kind="ExternalInput").ap()
    blin_in = nc.dram_tensor("blin_pad", [4, 1], F32, kind="ExternalInput").ap()
    tri_in = nc.dram_tensor("tri_r", [128, NCH, L], F32, kind="ExternalInput").ap()
    fr_in = nc.dram_tensor("frames", [1, T], F32, kind="ExternalInput").ap()
    iota_in = nc.dram_tensor("iota_c", [128, NCH], F32, kind="ExternalInput").ap()
    onesL_in = nc.dram_tensor("ones_L", [1, L], F32, kind="ExternalInput").ap()

    ups_out = nc.dram_tensor("ups_loc", [BPC, T, D], F32, kind="ExternalOutput").ap()
    wgt_out = nc.dram_tensor("wgt_loc", [BPC, L, T], F32, kind="ExternalOutput").ap()

    NB = BPC * NCH  # 16 column-layout slots, free index f = c*BPC + b

    with tile.TileContext(nc) as tc, ExitStack() as ctx:
        const = ctx.enter_context(tc.tile_pool(name="const", bufs=1))
        small = ctx.enter_context(tc.tile_pool(name="small", bufs=1))
        sqp = ctx.enter_context(tc.tile_pool(name="sqp", bufs=2))
        probsp = ctx.enter_context(tc.tile_pool(name="probsp", bufs=6))
        xsp = ctx.enter_context(tc.tile_pool(name="xsp", bufs=BPC * NCH))
        xstg = ctx.enter_context(tc.tile_pool(name="xstg", bufs=3))
        upsb = ctx.enter_context(tc.tile_pool(name="upsb", bufs=3))
        ps_big = ctx.enter_context(tc.tile_pool(name="ps_big", bufs=2, space="PSUM"))
        ps_small = ctx.enter_context(tc.tile_pool(name="ps_small", bufs=2, space="PSUM"))
        ps_eins = ctx.enter_context(tc.tile_pool(name="ps_eins", bufs=2, space="PSUM"))

        # ==== phase 0: loads =================================================
        tri_t = const.tile([128, NCH * L], F32)
        nc.sync.dma_start(tri_t[:], tri_in.rearrange("p c l -> p (c l)"))
        fr_t = const.tile([1, T], F32)
        nc.sync.dma_start(fr_t[:], fr_in[:])
        iota_t = const.tile([128, NCH], F32)
        nc.sync.dma_start(iota_t[:], iota_in[:])
        wdb_t = const.tile([128, NCH * 4], F32)
        nc.sync.dma_start(wdb_t[:], wdb_in.rearrange("p c k -> p (c k)"))
        wlinc_t = const.tile([128, NCH], F32)
        nc.sync.dma_start(wlinc_t[:], wlinc_in[:])
        wlinr_t = const.tile([1, D], F32)
        nc.sync.dma_start(wlinr_t[:], wlinr_in[:])
        blin_t = const.tile([4, 1], F32)
        nc.sync.dma_start(blin_t[:], blin_in[:])
        ben_t = const.tile([1, D], F32)
        nc.sync.dma_start(ben_t[:], ben_in[:])
        bpi_t = const.tile([1, D], F32)
        nc.sync.dma_start(bpi_t[:], bpi_in[:])
        lens_t = const.tile([1, BPC], I32)
        nc.sync.dma_start(lens_t[:], lens_in[:])

        ones_r = const.tile([1, 128], F32)
        nc.vector.memset(ones_r[:], 1.0)
        # f32r "ones" must come from a rounding compute op, not memset/DMA
        ones_cr = const.tile([128, 1], F32R)
        ones_c = const.tile([128, 1], F32)
        nc.vector.memset(ones_c[:], 1.0)
        nc.vector.tensor_copy(ones_cr[:], ones_c[:])
        ones_rr = const.tile([1, 128], F32R)
        nc.vector.tensor_copy(ones_rr[:], ones_r[:])

        # w7: rows 0-5 conv weights, row 6 = b_en + b_pi (via DRAM bounce for
        # the partition move)
        w7_t = const.tile([7, D], F32)
        nc.sync.dma_start(w7_t[0:6, :], w6_in[:])
        bsum_t = small.tile([1, D], F32, tag="bsum")
        nc.vector.tensor_add(bsum_t[:], ben_t[:], bpi_t[:])
        bsum_dram = nc.dram_tensor("bsum_bounce", [1, D], F32).ap()
        nc.sync.dma_start(bsum_dram[:], bsum_t[:])
        nc.sync.dma_start(w7_t[6:7, :], bsum_dram[:])

        # signal matrices: S7 (en/pi shifts + ones) and S4 (df shifts + ones)
        s7 = []
        s4 = []
        for b in range(BPC):
            s7_t = const.tile([7, L], F32, tag=f"s7_{b}")
            for s in range(3):
                nc.sync.dma_start(s7_t[s:s + 1, :], en_in[b:b + 1, s:s + L])
                nc.sync.dma_start(s7_t[3 + s:4 + s, :], pi_in[b:b + 1, s:s + L])
            nc.sync.dma_start(s7_t[6:7, :], onesL_in[:])
            s7.append(s7_t)
            s4_t = const.tile([4, L], F32, tag=f"s4_{b}")
            for s in range(3):
                nc.sync.dma_start(s4_t[s:s + 1, :], df_in[b:b + 1, s:s + L])
            nc.sync.dma_start(s4_t[3:4, :], onesL_in[:])
            s4.append(s4_t)

        # durations in column layout [p, (c b)]
        di_t = const.tile([128, NB], I32)
        for c in range(NCH):
            nc.sync.dma_start(di_t[:, _ts(c, BPC)],
                              di_in[:, _ts(c, 128)].rearrange("b p -> p b"))

        # ==== phase 0: small compute ========================================
        # lens broadcast to partitions: [p, b] = len_b
        lensf_t = small.tile([1, BPC], F32, tag="lensf")
        nc.vector.tensor_copy(lensf_t[:], lens_t[:])
        lens_ps = ps_small.tile([128, BPC], F32, tag="sm")
        nc.tensor.matmul(lens_ps[:], ones_r[:], lensf_t[:], start=True, stop=True)

        # valid[p, f] = iota_c < len_b
        valid_t = small.tile([128, NB], F32, tag="valid")
        for c in range(NCH):
            for b in range(BPC):
                f = c * BPC + b
                nc.vector.tensor_scalar(
                    valid_t[:, f:f + 1], iota_t[:, c:c + 1],
                    lens_ps[:, b:b + 1], None, OP.is_lt)

        # d columns, cumsum, negm = 0.5*d - cumsum
        dcol_t = small.tile([128, NB], F32, tag="dcol")
        nc.vector.tensor_copy(dcol_t[:], di_t[:])
        halfd_t = small.tile([128, NB], F32, tag="halfd")
        nc.vector.tensor_scalar(halfd_t[:], dcol_t[:], 0.5, None, OP.mult)
        negm_t = small.tile([128, NB], F32, tag="negm")
        for mc in range(NCH):
            cs_ps = ps_small.tile([128, BPC], F32, tag="sm")
            for kc in range(mc + 1):
                nc.tensor.matmul(
                    cs_ps[:],
                    tri_t[:, kc * L + mc * 128: kc * L + mc * 128 + 128],
                    dcol_t[:, _ts(kc, BPC)],
                    start=(kc == 0), stop=(kc == mc))
            nc.vector.tensor_sub(
                negm_t[:, _ts(mc, BPC)], halfd_t[:, _ts(mc, BPC)], cs_ps[:])

        # dur-conv coefficients: coef4 = [wdur.T @ wlin (3), bdur @ wlin + blin]
        coef_ps = ps_small.tile([4, 1], F32, tag="sm")
        for kc in range(NCH):
            nc.tensor.matmul(coef_ps[:], wdb_t[:, kc * 4:kc * 4 + 4],
                             wlinc_t[:, kc:kc + 1],
                             start=(kc == 0), stop=(kc == NCH - 1))
        coef_t = small.tile([4, 1], F32, tag="coef")
        nc.vector.tensor_add(coef_t[:], coef_ps[:], blin_t[:])

        # w_lin broadcast across partitions (for the free-dim dot with xs)
        wlb_ps = ps_big.tile([128, D], F32, tag="big")
        nc.tensor.matmul(wlb_ps[:], ones_r[:], wlinr_t[:], start=True, stop=True)
        wlb_t = const.tile([128, D], F32)
        nc.scalar.copy(wlb_t[:], wlb_ps[:])

        # frames broadcast across partitions
        fb_t = const.tile([128, T], F32)
        for (s, w) in tts:
            fb_ps = ps_big.tile([128, 512], F32, tag="big")
            nc.tensor.matmul(fb_ps[:, 0:w], ones_r[:], fr_t[:, s:s + w],
                             start=True, stop=True)
            nc.scalar.copy(fb_t[:, s:s + w], fb_ps[:, 0:w])

        # stage A: xs = x + S7.T @ w7 ; lin = xs.wlin + S4.T @ coef4 (+ pens)
        lin_t = small.tile([128, NB], F32, tag="lin")
        xs = {}
        for b in range(BPC):
            for c in range(NCH):
                f = c * BPC + b
                xs_ps = ps_big.tile([128, D], F32, tag="big")
                nc.tensor.matmul(xs_ps[:], s7[b][:, _ts(c, 128)], w7_t[:],
                                 start=True, stop=True)
                x_t = xstg.tile([128, D], F32)
                nc.sync.dma_start(x_t[:], x_in[b, _ts(c, 128), :])
                xs_t = xstg.tile([128, D], F32, tag="xsf")
                nc.vector.tensor_add(xs_t[:], xs_ps[:], x_t[:])
                xsr_t = xsp.tile([128, D], F32R, tag="xs")
                nc.scalar.copy(xsr_t[:], xs_t[:])
                xs[(b, c)] = xsr_t
                prod_t = xstg.tile([128, D], F32, tag="prod")
                nc.vector.tensor_mul(prod_t[:], xs_t[:], wlb_t[:])
                nc.vector.reduce_sum(
                    lin_t[:, f:f + 1], prod_t[:], axis=AX.X)
        for c in range(NCH):
            conv_ps = ps_small.tile([128, BPC], F32, tag="sm")
            for b in range(BPC):
                nc.tensor.matmul(conv_ps[:, b:b + 1], s4[b][:, _ts(c, 128)],
                                 coef_t[:], start=True, stop=True)
            nc.vector.tensor_add(lin_t[:, _ts(c, BPC)], lin_t[:, _ts(c, BPC)],
                                 conv_ps[:])
        # invalid rows: push lin to +1e4 so r is huge and harmless
        pen_t = small.tile([128, NB], F32, tag="pen")
        nc.vector.tensor_scalar(pen_t[:], valid_t[:], -1e4, 1e4, OP.mult, OP.add)
        nc.vector.tensor_add(lin_t[:], lin_t[:], pen_t[:])

        # r = softplus(lin) = Ln(Exp(lin)+1); a = -0.5/r^2 ;
        # c = -ln r - 0.5*log(2pi) - 1e30*(1-valid)
        e_t = small.tile([128, NB], F32, tag="e")
        nc.scalar.activation(e_t[:], lin_t[:], AF.Exp)
        r_t = small.tile([128, NB], F32, tag="r")
        nc.scalar.activation(r_t[:], e_t[:], AF.Ln, bias=1.0, scale=1.0)
        rinv_t = small.tile([128, NB], F32, tag="rinv")
        nc.vector.reciprocal(rinv_t[:], r_t[:])
        a_t = small.tile([128, NB], F32, tag="a")
        nc.vector.tensor_mul(a_t[:], rinv_t[:], rinv_t[:])
        nc.vector.tensor_scalar(a_t[:], a_t[:], -0.5, None, OP.mult)
        lnr_t = small.tile([128, NB], F32, tag="lnr")
        nc.scalar.activation(lnr_t[:], r_t[:], AF.Ln)
        c_t = small.tile([128, NB], F32, tag="c")
        nc.vector.tensor_scalar(c_t[:], lnr_t[:], -1.0, -0.5 * LOG_2PI,
                                OP.mult, OP.add)
        pen30_t = small.tile([128, NB], F32, tag="pen30")
        nc.vector.tensor_scalar(pen30_t[:], valid_t[:], 1e30, -1e30,
                                OP.mult, OP.add)
        nc.vector.tensor_add(c_t[:], c_t[:], pen30_t[:])

        # ==== main loop over batches ========================================
        for b in range(BPC):
            probs = []
            for c in range(NCH):
                f = c * BPC + b
                sq_t = sqp.tile([128, T], F32, tag="sq")
                nc.scalar.activation(sq_t[:], fb_t[:], AF.Square,
                                     bias=negm_t[:, f:f + 1], scale=1.0)
                p_t = probsp.tile([128, T], F32R, tag="probs")
                nc.scalar.activation(p_t[:], sq_t[:], AF.Exp,
                                     bias=c_t[:, f:f + 1],
                                     scale=a_t[:, f:f + 1])
                probs.append(p_t)

            # denominator over L (partition axis) via ones-column matmuls
            den_t = small.tile([1, T], F32, tag="den")
            for (s, w) in tts:
                den_ps = ps_small.tile([1, 512], F32, tag="den")
                for c in range(NCH):
                    nc.tensor.matmul(den_ps[:, 0:w], ones_cr[:],
                                     probs[c][:, s:s + w],
                                     start=(c == 0), stop=(c == NCH - 1))
                nc.scalar.activation(den_t[:, s:s + w], den_ps[:, 0:w],
                                     AF.Copy, bias=1e-20)
            rec_t = small.tile([1, T], F32R, tag="rec")
            with nc.allow_low_precision(reason="f32r feed for broadcast matmul"):
                nc.vector.reciprocal(rec_t[:], den_t[:])

            # weights = probs * recip (broadcast recip across partitions)
            for (s, w) in tts:
                rbc_ps = ps_big.tile([128, 512], F32, tag="big")
                nc.tensor.matmul(rbc_ps[:, 0:w], ones_rr[:],
                                 rec_t[:, s:s + w], start=True, stop=True)
                for c in range(NCH):
                    nc.vector.tensor_tensor(
                        probs[c][:, s:s + w], probs[c][:, s:s + w],
                        rbc_ps[:, 0:w], OP.mult)
            for c in range(NCH):
                nc.sync.dma_start(wgt_out[b, _ts(c, 128), :],
                                  probs[c][:].bitcast(F32))

            # x_upsamp = weights.T @ xs
            for (s, w) in mms:
                ups_ps = ps_eins.tile([128, D], F32, tag="eins")
                for c in range(NCH):
                    nc.tensor.matmul(ups_ps[0:w, :], probs[c][:, s:s + w],
                                     xs[(b, c)][:],
                                     start=(c == 0), stop=(c == NCH - 1))
                u_t = upsb.tile([128, D], F32, tag="ups")
                if s % 256 == 0:
                    nc.vector.tensor_copy(u_t[0:w, :], ups_ps[0:w, :])
                else:
                    nc.scalar.copy(u_t[0:w, :], ups_ps[0:w, :])
                nc.sync.dma_start(ups_out[b, s:s + w, :], u_t[0:w, :])

    nc.compile()
    return nc


def prep(inputs):
    """Build (cached) nc and the per-core input maps."""
    x = np.ascontiguousarray(np.asarray(inputs["x"], dtype=np.float32))
    durations_float = np.asarray(inputs["durations_float"], dtype=np.float32)
    durations_int = np.asarray(inputs["durations_int"], dtype=np.int32)
    energies = np.asarray(inputs["energies"], dtype=np.float32)
    pitch = np.asarray(inputs["pitch"], dtype=np.float32)
    input_lengths = np.asarray(inputs["input_lengths"], dtype=np.int32)
    w_dur = np.asarray(inputs["w_dur"], dtype=np.float32)
    b_dur = np.asarray(inputs["b_dur"], dtype=np.float32)
    w_en = np.asarray(inputs["w_en"], dtype=np.float32)
    b_en = np.asarray(inputs["b_en"], dtype=np.float32)
    w_pitch = np.asarray(inputs["w_pitch"], dtype=np.float32)
    b_pitch = np.asarray(inputs["b_pitch"], dtype=np.float32)
    w_lin = np.asarray(inputs["w_lin"], dtype=np.float32)
    b_lin = np.asarray(inputs["b_lin"], dtype=np.float32)
    T = int(inputs["nb_frames"])

    if T not in _NC_CACHE:
        _NC_CACHE[T] = _build(T)
    nc = _NC_CACHE[T]

    def pad2(a):  # (BPC, L) -> (BPC, L+2) zero-padded on both ends
        out = np.zeros((BPC, L + 2), dtype=np.float32)
        out[:, 1:L + 1] = a
        return np.ascontiguousarray(out)

    # constants (input-independent index/mask matrices)
    tri = np.triu(np.ones((L, L), dtype=np.float32))           # tri[k, m] = k<=m
    tri_r = np.ascontiguousarray(tri.reshape(NCH, 128, L).transpose(1, 0, 2))
    frames = (np.arange(T, dtype=np.float32) + 0.5)[None, :]
    iota_c = np.ascontiguousarray(
        np.arange(L, dtype=np.float32).reshape(NCH, 128).T)

    shared = dict(
        w6=np.ascontiguousarray(
            np.concatenate([w_en[:, 0, :].T, w_pitch[:, 0, :].T], axis=0)),
        ben=np.ascontiguousarray(b_en[None, :]),
        bpi=np.ascontiguousarray(b_pitch[None, :]),
        wdb_c=np.ascontiguousarray(np.concatenate(
            [w_dur[:, 0, :], b_dur[:, None]], axis=1
        ).reshape(NCH, 128, 4).transpose(1, 0, 2)),
        wlin_c=np.ascontiguousarray(w_lin[0].reshape(NCH, 128).T),
        wlin_r=np.ascontiguousarray(w_lin),
        blin_pad=np.array([[0.0], [0.0], [0.0], [float(b_lin[0])]],
                          dtype=np.float32),
        tri_r=tri_r,
        frames=np.ascontiguousarray(frames),
        iota_c=iota_c,
        ones_L=np.ones((1, L), dtype=np.float32),
    )

    in_maps = []
    for i in range(NCORES):
        sl = slice(i * BPC, (i + 1) * BPC)
        in_maps.append(dict(
            x_loc=np.ascontiguousarray(x[sl]),
            en_pad=pad2(energies[sl]),
            pi_pad=pad2(pitch[sl]),
            df_pad=pad2(durations_float[sl]),
            di_loc=np.ascontiguousarray(durations_int[sl]),
            lens_loc=np.ascontiguousarray(input_lengths[sl][None, :]),
            **shared,
        ))

    return nc, in_maps


def assemble(results):
    ups = np.concatenate([r["ups_loc"] for r in results], axis=0)
    wgt = np.concatenate([r["wgt_loc"] for r in results], axis=0)
    return ups, wgt


def kernel(**inputs):
    nc, in_maps = prep(inputs)
    res = run_bass_kernel_spmd(nc, in_maps, list(range(NCORES)))
    return assemble(res.results)
